# revision 16
# baseline (speedup 1.0000x reference)
"""Distributed causal self-attention kernel for Trainium2 (8 NeuronCores).

Sharding: batch x head-group grid (core c = 2*b + g: batch b, head group g of
8 heads = 512 channels). Host sums the two partial outputs per batch.

v3.1 (from the ~279us v2 baseline):
  - q/k projections run fp8e4m3 DoubleRow (256-row contraction per matmul,
    half the projection matmuls). Weights are host-scaled x64 so e4m3's
    mantissa sits in its sweet spot; the 4096x score scale folds into the
    exp's scale immediate. q/k quantization error reaches the output only
    through the softmax, which renormalizes it away (~1.1% measured).
  - v / probabilities / av / output projection stay bf16: fp8 v or p passes
    quantization error straight through concentrated softmax rows (measured
    2.7% rel err > the 2e-2 gate).
  - All psum->sbuf copies are pinned to the vector engine: the scheduler
    otherwise parks ~44us of them on the scalar engine, which is the
    co-bottleneck (exp stream).
  - Attention structure unchanged from v2: row-tiled score pairs, one exp
    per key chunk covering both heads, gpsimd affine_select causal masking,
    att @ v_aug with the [v|1]/[1|v] parity trick so the softmax denominator
    accumulates in the opposite 64-partition half at full PE width.
  - PSUM budget (8 banks): scores 2x2, y-accumulators 2, proj/outproj 2.

Layouts (host pre-transposes; contraction dim on partitions):
  xT [C, T] bf16 (v-proj lhsT)      xT8 [C, T] fp8 (q/k rhs)
  wqT8/wkT8 [C, 512] fp8 (x64)      wvT [C, 512] bf16
  wpT [512, C] bf16                 out [T, C] bf16 (partial; host sums)
"""

import sys

if "/opt/trn_rl_repo" not in sys.path:
    sys.path.insert(0, "/opt/trn_rl_repo")

from contextlib import ExitStack

import ml_dtypes
import numpy as np

import concourse.bass as bass
import concourse.mybir as mybir
import concourse.tile as tile
from concourse import bacc
from concourse.bass_utils import run_bass_kernel_spmd

B, T, C, H, D = 4, 2048, 1024, 16, 64
N_CORES = 8
HL = 8          # heads per core
CL = HL * D     # channels per core = 512
NCH = C // 128  # contraction chunks = 8
QBS = 512       # query block size
NQB = T // QBS  # query blocks = 4 (also token blocks)
KCS = 128       # key chunk size
QSC = 64.0      # fp8 weight scale for wq/wk (folded out in the exp scale)
F32 = mybir.dt.float32
BF16 = mybir.dt.bfloat16
F8 = mybir.dt.float8e4
DR = mybir.MatmulPerfMode.DoubleRow


def build_attn(ctx: ExitStack, tc: tile.TileContext, xT, xT8, wqT8, wkT8, wvT,
               wpT, out):
    nc = tc.nc
    Exp = mybir.ActivationFunctionType.Exp

    persist = ctx.enter_context(tc.tile_pool(name="persist", bufs=1))
    psum = ctx.enter_context(tc.tile_pool(name="psum", bufs=1, space="PSUM"))
    work = ctx.enter_context(tc.tile_pool(name="work", bufs=3))

    # ---- stage inputs in SBUF: serial DMAs on the sync queue, ordered by
    # first use. Staging is HBM-bandwidth-bound, so parallel queues only
    # interleave transfers and make the earliest-needed tensor land later
    # (measured +10us); serial in dependency order is optimal. ----
    def stage_dma(t, src_ap):
        nc.sync.dma_start(out=t, in_=src_ap)
        return t

    def stage(name, src, nch, cols, dt):
        t = persist.tile([128, nch, cols], dt, name=name)
        return stage_dma(t, src.rearrange("(c p) m -> p c m", p=128))

    # first block of wq8/x8 arrives as two 4-chunk halves so the first
    # projection starts early and its later chunks land before the matmuls
    # catch up (no mid-group stall)
    wq_a = stage_dma(persist.tile([128, 4, CL], F8, name="wq_a"),
                     wqT8[0:512, :].rearrange("(c p) m -> p c m", p=128))
    xt80_a = stage_dma(persist.tile([128, 4, QBS], F8, name="xt80_a"),
                       xT8[0:512, 0:QBS].rearrange("(c p) m -> p c m", p=128))
    wq_b = stage_dma(persist.tile([128, 4, CL], F8, name="wq_b"),
                     wqT8[512:C, :].rearrange("(c p) m -> p c m", p=128))
    xt80_b = stage_dma(persist.tile([128, 4, QBS], F8, name="xt80_b"),
                       xT8[512:C, 0:QBS].rearrange("(c p) m -> p c m", p=128))
    wk_all = stage("wk", wkT8, NCH, CL, F8)

    def stage_xbf(b):
        t = persist.tile([128, NCH, QBS], BF16, name=f"xt_{b}")
        return stage_dma(
            t, xT[:, b * QBS:(b + 1) * QBS].rearrange("(c p) m -> p c m",
                                                      p=128))

    xt_all = [stage_xbf(0)]
    wv_all = stage("wv", wvT, NCH, CL, BF16)
    xt8_all = [None]
    for b in range(1, NQB):
        t = persist.tile([128, NCH, QBS], F8, name=f"xt8_{b}")
        stage_dma(t, xT8[:, b * QBS:(b + 1) * QBS].rearrange(
            "(c p) m -> p c m", p=128))
        xt8_all.append(t)
    for b in range(1, NQB):
        xt_all.append(stage_xbf(b))
    wp_all = stage("wp", wpT, CL // 128, C, BF16)

    wq_sb = [wq_a, wq_a, wq_b, wq_b]       # chunk-pair cp -> tile + base idx
    wq_ix = [0, 2, 0, 2]
    wv_sb = [wv_all[:, c, :] for c in range(NCH)]
    wp_sb = [wp_all[:, i, :] for i in range(CL // 128)]

    def xt8_pair(b, cp):
        if b == 0:
            t, i = (xt80_a, 2 * cp) if cp < 2 else (xt80_b, 2 * cp - 4)
            return t[:, i:i + 2, :]
        return xt8_all[b][:, 2 * cp:2 * cp + 2, :]

    zero_fill = nc.gpsimd.to_reg(0.0)
    # warm the Exp activation table (~2.7us load) off the critical path
    warm = persist.tile([128, 1], F32, name="warm")
    nc.vector.memset(warm, 0.0)
    nc.scalar.activation(out=warm, in_=warm, func=Exp)

    # persistent projection outputs (q/k are 64x-scaled from the fp8 weights)
    qT_sb = [persist.tile([128, T], BF16, name=f"qT{i}") for i in range(4)]
    kT_sb = [persist.tile([128, T], BF16, name=f"kT{i}") for i in range(4)]
    # v_aug [key, head, 128]: even head h -> [v_h | 1], odd head -> [1 | v_h];
    # att @ v_aug then yields y in one 64-partition half and the softmax
    # denominator l (ones-columns) in the other, full-width on the PE.
    v_sb = [persist.tile([128, HL, 128], BF16, name=f"v{t}")
            for t in range(T // KCS)]
    for t in range(T // KCS):
        nc.vector.memset(v_sb[t][:, 0:HL:2, 64:128], 1.0)
        nc.vector.memset(v_sb[t][:, 1:HL:2, 0:64], 1.0)
    ytall = [[persist.tile([128, QBS], BF16, name=f"ytall{qb}_{cc}")
              for cc in range(4)] for qb in range(NQB)]

    def proj_block(b):
        """Project token block b: all qT/kT chains first (they only need the
        fp8 inputs, which land first), then the 4 v row chunks — the PE queue
        is in-order, so a v chain waiting on the bf16-x DMA must never sit
        ahead of runnable q/k work."""
        for i in range(4):
            for w_sb, w_ix, dst in ((wq_sb, wq_ix, qT_sb),
                                    ([wk_all] * 4, [0, 2, 4, 6], kT_sb)):
                pq = psum.tile([128, QBS], F32, name="pq", tag="pw", bufs=2)
                for cp in range(4):
                    wt = w_sb[cp]
                    nc.tensor.matmul(
                        pq,
                        lhsT=wt[:, w_ix[cp]:w_ix[cp] + 2,
                                i * 128:(i + 1) * 128],
                        rhs=xt8_pair(b, cp),
                        start=(cp == 0),
                        stop=(cp == 3),
                        perf_mode=DR,
                    )
                nc.vector.tensor_copy(out=dst[i][:, b * QBS:(b + 1) * QBS],
                                      in_=pq)
        for t in range(4 * b, 4 * b + 4):
            pv = psum.tile([128, CL], F32, name="pv", tag="pw", bufs=2)
            for c in range(NCH):
                nc.tensor.matmul(
                    pv,
                    lhsT=xt_all[b][:, c, (t % 4) * 128:(t % 4 + 1) * 128],
                    rhs=wv_sb[c],
                    start=(c == 0),
                    stop=(c == NCH - 1),
                )
            pv_h = pv.rearrange("p (h d) -> p h d", h=HL)
            nc.vector.tensor_copy(out=v_sb[t][:, 0:HL:2, 0:64],
                                  in_=pv_h[:, 0:HL:2, :])
            nc.vector.tensor_copy(out=v_sb[t][:, 1:HL:2, 64:128],
                                  in_=pv_h[:, 1:HL:2, :])

    AVL_DELAY = 4  # av matmuls trail scores by this many key chunks

    def attention(qb, hook=None):
        nkc = (qb + 1) * (QBS // KCS)
        for ht in range(4):
            h0, h1 = 2 * ht, 2 * ht + 1
            yt0 = psum.tile([128, QBS], F32, name="yt0", tag="yt", bufs=2)
            yt1 = psum.tile([128, QBS], F32, name="yt1", tag="yt", bufs=2)
            pend = {}

            def avl(kc):
                s, pt = pend.pop(kc)
                for j, yt, h in ((0, yt0, h0), (1, yt1, h1)):
                    nc.tensor.matmul(
                        yt[:, s:QBS],
                        lhsT=v_sb[kc][:, h, :],
                        rhs=pt[:, j, s:QBS],
                        start=(kc == 0),
                        stop=(kc == nkc - 1),
                    )

            def score_exp(kc):
                d = kc - qb * (QBS // KCS)
                s = d * KCS if d >= 0 else 0
                stp = psum.tile([128, 2, QBS], F32, name="stp", tag="stp",
                                bufs=2)
                for j in range(2):
                    nc.tensor.matmul(
                        stp[:, j, s:QBS],
                        lhsT=kT_sb[ht][64 * j:64 * j + 64,
                                       kc * KCS:(kc + 1) * KCS],
                        rhs=qT_sb[ht][64 * j:64 * j + 64,
                                      qb * QBS + s:(qb + 1) * QBS],
                        start=True,
                        stop=True,
                    )
                pt = work.tile([128, 2, QBS], BF16, name="pt", tag="pt", bufs=9)
                nc.scalar.activation(out=pt[:, :, s:QBS], in_=stp[:, :, s:QBS],
                                     func=Exp,
                                     scale=1.0 / (np.sqrt(D) * QSC * QSC))
                if d >= 0:
                    for j in range(2):
                        # zero where query < key within the diagonal 128-block
                        nc.gpsimd.affine_select(
                            out=pt[:, j, s:s + KCS],
                            in_=pt[:, j, s:s + KCS],
                            compare_op=mybir.AluOpType.is_ge,
                            fill=zero_fill,
                            base=0,
                            pattern=[[1, KCS]],
                            channel_multiplier=-1,
                        )
                pend[kc] = (s, pt)

            # 4-chunk blocks: consecutive same-array-region matmuls pipeline
            # at stream rate, so batch the score pairs and the av chains and
            # pay fewer PE region-switch bubbles.
            for k0 in range(0, nkc, AVL_DELAY):
                for kc in range(k0, k0 + AVL_DELAY):
                    score_exp(kc)
                if k0 >= AVL_DELAY:
                    for kc in range(k0 - AVL_DELAY, k0):
                        avl(kc)
            for kc in range(max(0, nkc - AVL_DELAY), nkc):
                avl(kc)
            # h0's l sits in yt0[64:128], h1's in yt1[0:64]; gather both into
            # one full-128 tile so a single fast reciprocal covers them (the
            # custom-DVE reciprocal mis-executes on base-partition-64 windows),
            # then scale the psum-resident y halves directly. The very last
            # head of the kernel runs as two half-width chains so the final
            # outproj's first token tiles unblock ~1.4us sooner (the serial
            # DVE chain sits on the critical tail).
            halves = ((0, QBS),) if not (qb == NQB - 1 and ht == 3) \
                else ((0, QBS // 2), (QBS // 2, QBS))
            for a, bq in halves:
                lrec = work.tile([128, QBS], F32, name="lrec", tag="lrec",
                                 bufs=2)
                nc.vector.tensor_copy(out=lrec[0:64, a:bq],
                                      in_=yt0[64:128, a:bq])
                nc.vector.tensor_copy(out=lrec[64:128, a:bq],
                                      in_=yt1[0:64, a:bq])
                rec = work.tile([128, QBS], F32, name="rec", tag="rec", bufs=2)
                nc.vector.reciprocal_approx_fast(rec[:, a:bq], lrec[:, a:bq])
                nc.vector.tensor_mul(ytall[qb][ht][0:64, a:bq],
                                     yt0[0:64, a:bq], rec[0:64, a:bq])
                nc.vector.tensor_mul(ytall[qb][ht][64:128, a:bq],
                                     yt1[64:128, a:bq], rec[64:128, a:bq])
            if hook is not None:
                hook(ht)

    def outproj_tile(qb, tt, jt, po, ccs, start, stop):
        for n, cc in enumerate(ccs):
            nc.tensor.matmul(
                po,
                lhsT=ytall[qb][cc][:, tt * 128:(tt + 1) * 128],
                rhs=wp_sb[cc][:, jt * QBS:(jt + 1) * QBS],
                start=start and (n == 0),
                stop=stop and (n == len(ccs) - 1),
            )

    def outproj_emit(qb, tt, jt, po):
        ot = work.tile([128, QBS], BF16, name="ot", tag="ot", bufs=3)
        nc.vector.tensor_copy(out=ot, in_=po)
        nc.sync.dma_start(
            out=out[qb * QBS + tt * 128:qb * QBS + (tt + 1) * 128,
                    jt * QBS:(jt + 1) * QBS],
            in_=ot,
        )

    def outproj(qb, skip=()):
        for tt in range(QBS // 128):
            for jt in range(C // QBS):
                if (tt, jt) in skip:
                    continue
                po = psum.tile([128, QBS], F32, name="po", tag="pw", bufs=2)
                outproj_tile(qb, tt, jt, po, range(4), True, True)
                outproj_emit(qb, tt, jt, po)

    # emission (= scheduler priority) order: attention(qb) scores stay ahead
    # of the next projection block; outproj(qb) follows immediately so its
    # matmuls interleave with the next attention row instead of piling up
    # behind the final row as a cold tail. The last row's first outproj
    # tiles pre-accumulate cc=0..2 inside the ht==2 window, so after the
    # final norm only one matmul per tile remains on the critical tail.
    split3 = [(0, 0), (0, 1)]
    po3 = {}

    def att3_hook(ht):
        if ht != 2:
            return
        # outproj(2) goes here: late enough that row 3's exp pipeline is
        # already flowing, early enough to stay out of the tail. The po3
        # chains must be the ring's last users before their cc=3 finish.
        outproj(NQB - 2)
        for tt, jt in split3:
            po = psum.tile([128, QBS], F32, name="po3", tag="pw", bufs=2)
            outproj_tile(NQB - 1, tt, jt, po, range(3), True, False)
            po3[(tt, jt)] = po

    proj_block(0)
    for qb in range(NQB):
        attention(qb, hook=att3_hook if qb == NQB - 1 else None)
        if qb + 1 < NQB:
            proj_block(qb + 1)
        if 1 <= qb < NQB - 1:
            outproj(qb - 1)
    for tt, jt in split3:
        po = po3[(tt, jt)]
        outproj_tile(NQB - 1, tt, jt, po, [3], False, True)
        outproj_emit(NQB - 1, tt, jt, po)
    outproj(NQB - 1, skip=split3)


def _enable_ldw_opt():
    # the boot-time walrus flags carry --enable-ldw-opt=false, which forces a
    # serial LDWEIGHTS before every MATMUL (~107ns each); re-enable the opt
    from concourse.compiler_utils import get_compiler_flags, set_compiler_flags
    flags = [f.replace("--enable-ldw-opt=false", "--enable-ldw-opt=true")
             for f in get_compiler_flags()]
    set_compiler_flags(flags)


def _flatten_sched_pe_clock():
    # The Tile scheduler's cost sim models the PE p-state ramp (1.2GHz until
    # 3us of continuous busy). Our stream keeps the PE dense enough that the
    # hardware runs at full clock, so let the sim match — this only shapes
    # the static instruction order; correctness is semaphore-enforced.
    from concourse import hw_specs
    hw_specs.TRN2Spec.PE_CYCLE_PSTATE_LOW = hw_specs.TRN2Spec.PE_CYCLE
    hw_specs.TRN2Spec.PE_CYCLE_PSTATE_MID = hw_specs.TRN2Spec.PE_CYCLE


def build_nc():
    _enable_ldw_opt()
    _flatten_sched_pe_clock()
    nc = bacc.Bacc("TRN2", target_bir_lowering=False, debug=False,
                   enable_asserts=False, num_devices=N_CORES)
    xT = nc.dram_tensor("xT", [C, T], BF16, kind="ExternalInput").ap()
    xT8 = nc.dram_tensor("xT8", [C, T], F8, kind="ExternalInput").ap()
    wqT8 = nc.dram_tensor("wqT8", [C, CL], F8, kind="ExternalInput").ap()
    wkT8 = nc.dram_tensor("wkT8", [C, CL], F8, kind="ExternalInput").ap()
    wvT = nc.dram_tensor("wvT", [C, CL], BF16, kind="ExternalInput").ap()
    wpT = nc.dram_tensor("wpT", [CL, C], BF16, kind="ExternalInput").ap()
    out = nc.dram_tensor("out", [T, C], BF16, kind="ExternalOutput").ap()
    with tile.TileContext(nc) as tc:
        with ExitStack() as ctx:
            build_attn(ctx, tc, xT, xT8, wqT8, wkT8, wvT, wpT, out)
    nc.compile()
    return nc


_NC = None


def get_nc():
    global _NC
    if _NC is None:
        _NC = build_nc()
    return _NC


def make_in_maps(x, Wq, Wk, Wv, Wp):
    bf = ml_dtypes.bfloat16
    f8 = ml_dtypes.float8_e4m3fn
    in_maps = []
    for b in range(B):
        xT_b = np.ascontiguousarray(np.asarray(x[b]).T)
        xT_bf = xT_b.astype(bf)
        xT_f8 = xT_b.astype(f8)
        for g in range(2):
            sl = slice(g * CL, (g + 1) * CL)
            in_maps.append({
                "xT": xT_bf,
                "xT8": xT_f8,
                "wqT8": np.ascontiguousarray(
                    np.asarray(Wq)[sl, :].T * QSC).astype(f8),
                "wkT8": np.ascontiguousarray(
                    np.asarray(Wk)[sl, :].T * QSC).astype(f8),
                "wvT": np.ascontiguousarray(np.asarray(Wv)[sl, :].T).astype(bf),
                "wpT": np.ascontiguousarray(np.asarray(Wp)[:, sl].T).astype(bf),
            })
    return in_maps


def kernel(x, Wq, Wk, Wv, Wp):
    nc = get_nc()
    in_maps = make_in_maps(x, Wq, Wk, Wv, Wp)
    res = run_bass_kernel_spmd(nc, in_maps, list(range(N_CORES)))
    out = np.empty((B, T, C), dtype=np.float32)
    for b in range(B):
        out[b] = (res.results[2 * b]["out"].astype(np.float32)
                  + res.results[2 * b + 1]["out"].astype(np.float32))
    return out


if __name__ == "__main__":
    rng = np.random.default_rng(0)
    ins = {
        "x": rng.standard_normal((B, T, C), dtype=np.float32),
        "Wq": (rng.standard_normal((C, C), dtype=np.float32) * 0.02),
        "Wk": (rng.standard_normal((C, C), dtype=np.float32) * 0.02),
        "Wv": (rng.standard_normal((C, C), dtype=np.float32) * 0.02),
        "Wp": (rng.standard_normal((C, C), dtype=np.float32) * 0.02),
    }
    got = kernel(**ins)
    print("kernel output", got.shape, got.dtype)


# revision 20
# speedup vs baseline: 1.0089x; 1.0089x over previous
"""Distributed causal self-attention kernel for Trainium2 (8 NeuronCores).

Sharding: batch x head-group grid (core c = 2*b + g: batch b, head group g of
8 heads = 512 channels). Host sums the two partial outputs per batch.

v3.1 (from the ~279us v2 baseline):
  - q/k projections run fp8e4m3 DoubleRow (256-row contraction per matmul,
    half the projection matmuls). Weights are host-scaled x64 so e4m3's
    mantissa sits in its sweet spot; the 4096x score scale folds into the
    exp's scale immediate. q/k quantization error reaches the output only
    through the softmax, which renormalizes it away (~1.1% measured).
  - v / probabilities / av / output projection stay bf16: fp8 v or p passes
    quantization error straight through concentrated softmax rows (measured
    2.7% rel err > the 2e-2 gate).
  - All psum->sbuf copies are pinned to the vector engine: the scheduler
    otherwise parks ~44us of them on the scalar engine, which is the
    co-bottleneck (exp stream).
  - Attention structure unchanged from v2: row-tiled score pairs, one exp
    per key chunk covering both heads, gpsimd affine_select causal masking,
    att @ v_aug with the [v|1]/[1|v] parity trick so the softmax denominator
    accumulates in the opposite 64-partition half at full PE width.
  - PSUM budget (8 banks): scores 2x2, y-accumulators 2, proj/outproj 2.

Layouts (host pre-transposes; contraction dim on partitions):
  xT [C, T] bf16 (v-proj lhsT)      xT8 [C, T] fp8 (q/k rhs)
  wqT8/wkT8 [C, 512] fp8 (x64)      wvT [C, 512] bf16
  wpT [512, C] bf16                 out [T, C] bf16 (partial; host sums)
"""

import sys

if "/opt/trn_rl_repo" not in sys.path:
    sys.path.insert(0, "/opt/trn_rl_repo")

from contextlib import ExitStack

import ml_dtypes
import numpy as np

import concourse.bass as bass
import concourse.mybir as mybir
import concourse.tile as tile
from concourse import bacc
from concourse.bass_utils import run_bass_kernel_spmd

B, T, C, H, D = 4, 2048, 1024, 16, 64
N_CORES = 8
HL = 8          # heads per core
CL = HL * D     # channels per core = 512
NCH = C // 128  # contraction chunks = 8
QBS = 512       # query block size
NQB = T // QBS  # query blocks = 4 (also token blocks)
KCS = 128       # key chunk size
QSC = 64.0      # fp8 weight scale for wq/wk (folded out in the exp scale)
F32 = mybir.dt.float32
BF16 = mybir.dt.bfloat16
F8 = mybir.dt.float8e4
DR = mybir.MatmulPerfMode.DoubleRow


def build_attn(ctx: ExitStack, tc: tile.TileContext, xT, xT8, wqT8, wkT8, wvT,
               wpT, out):
    nc = tc.nc
    Exp = mybir.ActivationFunctionType.Exp

    persist = ctx.enter_context(tc.tile_pool(name="persist", bufs=1))
    psum = ctx.enter_context(tc.tile_pool(name="psum", bufs=1, space="PSUM"))
    work = ctx.enter_context(tc.tile_pool(name="work", bufs=3))

    # ---- stage inputs in SBUF: serial DMAs on the sync queue, ordered by
    # first use. Staging is HBM-bandwidth-bound, so parallel queues only
    # interleave transfers and make the earliest-needed tensor land later
    # (measured +10us); serial in dependency order is optimal. ----
    def stage_dma(t, src_ap):
        nc.sync.dma_start(out=t, in_=src_ap)
        return t

    def stage(name, src, nch, cols, dt):
        t = persist.tile([128, nch, cols], dt, name=name)
        return stage_dma(t, src.rearrange("(c p) m -> p c m", p=128))

    # first block of wq8/x8 arrives as two 4-chunk halves so the first
    # projection starts early and its later chunks land before the matmuls
    # catch up (no mid-group stall)
    wq_a = stage_dma(persist.tile([128, 4, CL], F8, name="wq_a"),
                     wqT8[0:512, :].rearrange("(c p) m -> p c m", p=128))
    xt80_a = stage_dma(persist.tile([128, 4, QBS], F8, name="xt80_a"),
                       xT8[0:512, 0:QBS].rearrange("(c p) m -> p c m", p=128))
    wq_b = stage_dma(persist.tile([128, 4, CL], F8, name="wq_b"),
                     wqT8[512:C, :].rearrange("(c p) m -> p c m", p=128))
    xt80_b = stage_dma(persist.tile([128, 4, QBS], F8, name="xt80_b"),
                       xT8[512:C, 0:QBS].rearrange("(c p) m -> p c m", p=128))
    wk_all = stage("wk", wkT8, NCH, CL, F8)

    def stage_xbf(b):
        t = persist.tile([128, NCH, QBS], BF16, name=f"xt_{b}")
        return stage_dma(
            t, xT[:, b * QBS:(b + 1) * QBS].rearrange("(c p) m -> p c m",
                                                      p=128))

    xt_all = [stage_xbf(0)]
    wv_all = stage("wv", wvT, NCH, CL, BF16)
    xt8_all = [None]
    for b in range(1, NQB):
        t = persist.tile([128, NCH, QBS], F8, name=f"xt8_{b}")
        stage_dma(t, xT8[:, b * QBS:(b + 1) * QBS].rearrange(
            "(c p) m -> p c m", p=128))
        xt8_all.append(t)
    for b in range(1, NQB):
        xt_all.append(stage_xbf(b))
    wp_all = stage("wp", wpT, CL // 128, C, BF16)

    wq_sb = [wq_a, wq_a, wq_b, wq_b]       # chunk-pair cp -> tile + base idx
    wq_ix = [0, 2, 0, 2]
    wv_sb = [wv_all[:, c, :] for c in range(NCH)]
    wp_sb = [wp_all[:, i, :] for i in range(CL // 128)]

    def xt8_pair(b, cp):
        if b == 0:
            t, i = (xt80_a, 2 * cp) if cp < 2 else (xt80_b, 2 * cp - 4)
            return t[:, i:i + 2, :]
        return xt8_all[b][:, 2 * cp:2 * cp + 2, :]

    zero_fill = nc.gpsimd.to_reg(0.0)
    # warm the Exp activation table (~2.7us load) off the critical path
    warm = persist.tile([128, 1], F32, name="warm")
    nc.vector.memset(warm, 0.0)
    nc.scalar.activation(out=warm, in_=warm, func=Exp)

    # persistent projection outputs (q/k are 64x-scaled from the fp8 weights)
    qT_sb = [persist.tile([128, T], BF16, name=f"qT{i}") for i in range(4)]
    kT_sb = [persist.tile([128, T], BF16, name=f"kT{i}") for i in range(4)]
    # v_aug [key, head, 128]: even head h -> [v_h | 1], odd head -> [1 | v_h];
    # att @ v_aug then yields y in one 64-partition half and the softmax
    # denominator l (ones-columns) in the other, full-width on the PE.
    v_sb = [persist.tile([128, HL, 128], BF16, name=f"v{t}")
            for t in range(T // KCS)]
    for t in range(T // KCS):
        nc.vector.memset(v_sb[t][:, 0:HL:2, 64:128], 1.0)
        nc.vector.memset(v_sb[t][:, 1:HL:2, 0:64], 1.0)
    ytall = [[persist.tile([128, QBS], BF16, name=f"ytall{qb}_{cc}")
              for cc in range(4)] for qb in range(NQB)]

    def proj_block(b):
        """Project token block b: all qT/kT chains first (they only need the
        fp8 inputs, which land first), then the 4 v row chunks — the PE queue
        is in-order, so a v chain waiting on the bf16-x DMA must never sit
        ahead of runnable q/k work."""
        for i in range(4):
            for w_sb, w_ix, dst in ((wq_sb, wq_ix, qT_sb),
                                    ([wk_all] * 4, [0, 2, 4, 6], kT_sb)):
                pq = psum.tile([128, QBS], F32, name="pq", tag="pw", bufs=2)
                for cp in range(4):
                    wt = w_sb[cp]
                    nc.tensor.matmul(
                        pq,
                        lhsT=wt[:, w_ix[cp]:w_ix[cp] + 2,
                                i * 128:(i + 1) * 128],
                        rhs=xt8_pair(b, cp),
                        start=(cp == 0),
                        stop=(cp == 3),
                        perf_mode=DR,
                    )
                nc.vector.tensor_copy(out=dst[i][:, b * QBS:(b + 1) * QBS],
                                      in_=pq)
        for t in range(4 * b, 4 * b + 4):
            pv = psum.tile([128, CL], F32, name="pv", tag="pw", bufs=2)
            for c in range(NCH):
                nc.tensor.matmul(
                    pv,
                    lhsT=xt_all[b][:, c, (t % 4) * 128:(t % 4 + 1) * 128],
                    rhs=wv_sb[c],
                    start=(c == 0),
                    stop=(c == NCH - 1),
                )
            pv_h = pv.rearrange("p (h d) -> p h d", h=HL)
            nc.vector.tensor_copy(out=v_sb[t][:, 0:HL:2, 0:64],
                                  in_=pv_h[:, 0:HL:2, :])
            nc.vector.tensor_copy(out=v_sb[t][:, 1:HL:2, 64:128],
                                  in_=pv_h[:, 1:HL:2, :])

    AVL_DELAY = 6  # av matmuls trail scores by this many key chunks

    def attention(qb, hook=None):
        nkc = (qb + 1) * (QBS // KCS)
        for ht in range(4):
            h0, h1 = 2 * ht, 2 * ht + 1
            yt0 = psum.tile([128, QBS], F32, name="yt0", tag="yt", bufs=2)
            yt1 = psum.tile([128, QBS], F32, name="yt1", tag="yt", bufs=2)
            pend = {}

            def avl(kc):
                s, pt = pend.pop(kc)
                for j, yt, h in ((0, yt0, h0), (1, yt1, h1)):
                    nc.tensor.matmul(
                        yt[:, s:QBS],
                        lhsT=v_sb[kc][:, h, :],
                        rhs=pt[:, j, s:QBS],
                        start=(kc == 0),
                        stop=(kc == nkc - 1),
                    )

            def score_exp(kc):
                d = kc - qb * (QBS // KCS)
                s = d * KCS if d >= 0 else 0
                stp = psum.tile([128, 2, QBS], F32, name="stp", tag="stp",
                                bufs=2)
                for j in range(2):
                    nc.tensor.matmul(
                        stp[:, j, s:QBS],
                        lhsT=kT_sb[ht][64 * j:64 * j + 64,
                                       kc * KCS:(kc + 1) * KCS],
                        rhs=qT_sb[ht][64 * j:64 * j + 64,
                                      qb * QBS + s:(qb + 1) * QBS],
                        start=True,
                        stop=True,
                    )
                pt = work.tile([128, 2, QBS], BF16, name="pt", tag="pt", bufs=11)
                nc.scalar.activation(out=pt[:, :, s:QBS], in_=stp[:, :, s:QBS],
                                     func=Exp,
                                     scale=1.0 / (np.sqrt(D) * QSC * QSC))
                if d >= 0:
                    for j in range(2):
                        # zero where query < key within the diagonal 128-block
                        nc.gpsimd.affine_select(
                            out=pt[:, j, s:s + KCS],
                            in_=pt[:, j, s:s + KCS],
                            compare_op=mybir.AluOpType.is_ge,
                            fill=zero_fill,
                            base=0,
                            pattern=[[1, KCS]],
                            channel_multiplier=-1,
                        )
                pend[kc] = (s, pt)

            # 4-chunk blocks: consecutive same-array-region matmuls pipeline
            # at stream rate, so batch the score pairs and the av chains and
            # pay fewer PE region-switch bubbles.
            for k0 in range(0, nkc, AVL_DELAY):
                for kc in range(k0, min(k0 + AVL_DELAY, nkc)):
                    score_exp(kc)
                if k0 >= AVL_DELAY:
                    for kc in range(k0 - AVL_DELAY, k0):
                        avl(kc)
            for kc in sorted(pend):
                avl(kc)
            # h0's l sits in yt0[64:128], h1's in yt1[0:64]; gather both into
            # one full-128 tile so a single fast reciprocal covers them (the
            # custom-DVE reciprocal mis-executes on base-partition-64 windows),
            # then scale the psum-resident y halves directly. The very last
            # head of the kernel runs as two half-width chains so the final
            # outproj's first token tiles unblock ~1.4us sooner (the serial
            # DVE chain sits on the critical tail).
            halves = ((0, QBS),) if not (qb == NQB - 1 and ht == 3) \
                else ((0, QBS // 2), (QBS // 2, QBS))
            for a, bq in halves:
                lrec = work.tile([128, QBS], F32, name="lrec", tag="lrec",
                                 bufs=2)
                nc.vector.tensor_copy(out=lrec[0:64, a:bq],
                                      in_=yt0[64:128, a:bq])
                nc.vector.tensor_copy(out=lrec[64:128, a:bq],
                                      in_=yt1[0:64, a:bq])
                rec = work.tile([128, QBS], F32, name="rec", tag="rec", bufs=2)
                nc.vector.reciprocal_approx_fast(rec[:, a:bq], lrec[:, a:bq])
                nc.vector.tensor_mul(ytall[qb][ht][0:64, a:bq],
                                     yt0[0:64, a:bq], rec[0:64, a:bq])
                nc.vector.tensor_mul(ytall[qb][ht][64:128, a:bq],
                                     yt1[64:128, a:bq], rec[64:128, a:bq])
            if hook is not None:
                hook(ht)

    def outproj_tile(qb, tt, jt, po, ccs, start, stop):
        for n, cc in enumerate(ccs):
            nc.tensor.matmul(
                po,
                lhsT=ytall[qb][cc][:, tt * 128:(tt + 1) * 128],
                rhs=wp_sb[cc][:, jt * QBS:(jt + 1) * QBS],
                start=start and (n == 0),
                stop=stop and (n == len(ccs) - 1),
            )

    def outproj_emit(qb, tt, jt, po):
        ot = work.tile([128, QBS], BF16, name="ot", tag="ot", bufs=3)
        nc.vector.tensor_copy(out=ot, in_=po)
        nc.sync.dma_start(
            out=out[qb * QBS + tt * 128:qb * QBS + (tt + 1) * 128,
                    jt * QBS:(jt + 1) * QBS],
            in_=ot,
        )

    def outproj(qb, skip=()):
        for tt in range(QBS // 128):
            for jt in range(C // QBS):
                if (tt, jt) in skip:
                    continue
                po = psum.tile([128, QBS], F32, name="po", tag="pw", bufs=2)
                outproj_tile(qb, tt, jt, po, range(4), True, True)
                outproj_emit(qb, tt, jt, po)

    # emission (= scheduler priority) order: attention(qb) scores stay ahead
    # of the next projection block; outproj(qb) follows immediately so its
    # matmuls interleave with the next attention row instead of piling up
    # behind the final row as a cold tail. The last row's first outproj
    # tiles pre-accumulate cc=0..2 inside the ht==2 window, so after the
    # final norm only one matmul per tile remains on the critical tail.
    split3 = [(0, 0), (0, 1)]
    po3 = {}

    def att3_hook(ht):
        if ht != 2:
            return
        # outproj(2) goes here: late enough that row 3's exp pipeline is
        # already flowing, early enough to stay out of the tail. The po3
        # chains must be the ring's last users before their cc=3 finish.
        outproj(NQB - 2)
        for tt, jt in split3:
            po = psum.tile([128, QBS], F32, name="po3", tag="pw", bufs=2)
            outproj_tile(NQB - 1, tt, jt, po, range(3), True, False)
            po3[(tt, jt)] = po

    proj_block(0)
    for qb in range(NQB):
        attention(qb, hook=att3_hook if qb == NQB - 1 else None)
        if qb + 1 < NQB:
            proj_block(qb + 1)
        if 1 <= qb < NQB - 1:
            outproj(qb - 1)
    for tt, jt in split3:
        po = po3[(tt, jt)]
        outproj_tile(NQB - 1, tt, jt, po, [3], False, True)
        outproj_emit(NQB - 1, tt, jt, po)
    outproj(NQB - 1, skip=split3)


def _enable_ldw_opt():
    # the boot-time walrus flags carry --enable-ldw-opt=false, which forces a
    # serial LDWEIGHTS before every MATMUL (~107ns each); re-enable the opt
    from concourse.compiler_utils import get_compiler_flags, set_compiler_flags
    flags = [f.replace("--enable-ldw-opt=false", "--enable-ldw-opt=true")
             for f in get_compiler_flags()]
    set_compiler_flags(flags)


def _flatten_sched_pe_clock():
    # The Tile scheduler's cost sim models the PE p-state ramp (1.2GHz until
    # 3us of continuous busy). Our stream keeps the PE dense enough that the
    # hardware runs at full clock, so let the sim match — this only shapes
    # the static instruction order; correctness is semaphore-enforced.
    from concourse import hw_specs
    hw_specs.TRN2Spec.PE_CYCLE_PSTATE_LOW = hw_specs.TRN2Spec.PE_CYCLE
    hw_specs.TRN2Spec.PE_CYCLE_PSTATE_MID = hw_specs.TRN2Spec.PE_CYCLE


def build_nc():
    _enable_ldw_opt()
    nc = bacc.Bacc("TRN2", target_bir_lowering=False, debug=False,
                   enable_asserts=False, num_devices=N_CORES)
    xT = nc.dram_tensor("xT", [C, T], BF16, kind="ExternalInput").ap()
    xT8 = nc.dram_tensor("xT8", [C, T], F8, kind="ExternalInput").ap()
    wqT8 = nc.dram_tensor("wqT8", [C, CL], F8, kind="ExternalInput").ap()
    wkT8 = nc.dram_tensor("wkT8", [C, CL], F8, kind="ExternalInput").ap()
    wvT = nc.dram_tensor("wvT", [C, CL], BF16, kind="ExternalInput").ap()
    wpT = nc.dram_tensor("wpT", [CL, C], BF16, kind="ExternalInput").ap()
    out = nc.dram_tensor("out", [T, C], BF16, kind="ExternalOutput").ap()
    with tile.TileContext(nc) as tc:
        with ExitStack() as ctx:
            build_attn(ctx, tc, xT, xT8, wqT8, wkT8, wvT, wpT, out)
    nc.compile()
    return nc


_NC = None


def get_nc():
    global _NC
    if _NC is None:
        _NC = build_nc()
    return _NC


def make_in_maps(x, Wq, Wk, Wv, Wp):
    bf = ml_dtypes.bfloat16
    f8 = ml_dtypes.float8_e4m3fn
    in_maps = []
    for b in range(B):
        xT_b = np.ascontiguousarray(np.asarray(x[b]).T)
        xT_bf = xT_b.astype(bf)
        xT_f8 = xT_b.astype(f8)
        for g in range(2):
            sl = slice(g * CL, (g + 1) * CL)
            in_maps.append({
                "xT": xT_bf,
                "xT8": xT_f8,
                "wqT8": np.ascontiguousarray(
                    np.asarray(Wq)[sl, :].T * QSC).astype(f8),
                "wkT8": np.ascontiguousarray(
                    np.asarray(Wk)[sl, :].T * QSC).astype(f8),
                "wvT": np.ascontiguousarray(np.asarray(Wv)[sl, :].T).astype(bf),
                "wpT": np.ascontiguousarray(np.asarray(Wp)[:, sl].T).astype(bf),
            })
    return in_maps


def kernel(x, Wq, Wk, Wv, Wp):
    nc = get_nc()
    in_maps = make_in_maps(x, Wq, Wk, Wv, Wp)
    res = run_bass_kernel_spmd(nc, in_maps, list(range(N_CORES)))
    out = np.empty((B, T, C), dtype=np.float32)
    for b in range(B):
        out[b] = (res.results[2 * b]["out"].astype(np.float32)
                  + res.results[2 * b + 1]["out"].astype(np.float32))
    return out


if __name__ == "__main__":
    rng = np.random.default_rng(0)
    ins = {
        "x": rng.standard_normal((B, T, C), dtype=np.float32),
        "Wq": (rng.standard_normal((C, C), dtype=np.float32) * 0.02),
        "Wk": (rng.standard_normal((C, C), dtype=np.float32) * 0.02),
        "Wv": (rng.standard_normal((C, C), dtype=np.float32) * 0.02),
        "Wp": (rng.standard_normal((C, C), dtype=np.float32) * 0.02),
    }
    got = kernel(**ins)
    print("kernel output", got.shape, got.dtype)
